# revision 17
# baseline (speedup 1.0000x reference)
"""Trainium2 Bass kernel for a batched linear-chain CRF negative log-likelihood.

reference semantics (B=128, S=2048, T=128):
    forward algorithm over S steps (log-space matvec chain) -> log_Z per batch
    gold path score = emissions gathered at tags + transitions gathered at
    (tag_t, tag_{t+1}) pairs, summed over time
    output = mean(log_Z - seq_score)   (scalar f32)

Strategy:
  - data parallel over 8 cores: 16 batch rows per core, transitions replicated.
  - linear space: a_t = (a_{t-1} @ W) * E_t with W = exp(transitions),
    E_t = exp(emit_t - chat).  Per-step work: one PE matmul (stationary W,
    moving state [128 tags x 16 batch]) + one DVE multiply out of PSUM.
  - bidirectional: forward chain from t=0 and a backward chain
    y_t = E_t * (W @ y_{t+1}) from t=2047 run concurrently and meet at
    t=1023: log_Z = log(a_m . (W y_{m+1})) + accumulated log scales.
  - periodic renormalization (every 32 steps) keeps values in range; scale
    logs accumulate per batch.
  - E precomputed into a transposed [tag, t*16+b] bf16 buffer via PE
    transpose + scalar-engine exp evacuation (bias = -chat).
  - gold path on device: one-hot tiles (iota vs tag compare on gpsimd),
    pair-count matrix C_b = OH^T @ OHshift accumulated on PE,
    sum(C*trans) via DVE fused tensor_tensor_reduce; emission select via
    gpsimd multiply + scalar-engine copy-with-accumulate.
"""

import numpy as np

B, S, T = 128, 2048, 128
NCORES = 8
BC = B // NCORES  # 16 batch rows per core
NSB = S // 128  # 16 s-blocks of 128
MID = S // 2 - 1  # 1023: chains meet here
RENORM = 32
JUNK_TAG = 60000.0  # one-hot of this is all zeros (tags are < 128)

_compiled = None


def _build_program(do_chain=True, do_gold=True, nrot=None, gold_mode='all'):
    import concourse.bass as bass
    import concourse.bacc as bacc
    import concourse.tile as tile
    from concourse import mybir
    from concourse.masks import make_identity

    fp32 = mybir.dt.float32
    bf16 = mybir.dt.bfloat16
    AF = mybir.ActivationFunctionType
    ALU = mybir.AluOpType
    AX = mybir.AxisListType

    nc = bacc.Bacc(None)
    em_d = nc.declare_dram_parameter("emissions_sh", [BC, S, T], fp32, isOutput=False)
    tr_d = nc.declare_dram_parameter("transitions", [T, T], fp32, isOutput=False)
    tg_d = nc.declare_dram_parameter("tags_sh", [BC, S], mybir.dt.int32, isOutput=False)
    out_d = nc.declare_dram_parameter("loss_parts", [BC], fp32, isOutput=True)
    dbg_d = nc.declare_dram_parameter("dbg", [4 * BC], fp32, isOutput=True)

    with tile.TileContext(nc) as tc:
        with (
            tc.tile_pool(name="consts", bufs=1) as consts,
            tc.tile_pool(name="ebuf", bufs=1) as ebufp,
            tc.tile_pool(name="emis", bufs=4) as emisp,
            tc.tile_pool(name="emis2", bufs=4) as emis2p,
            tc.tile_pool(name="oh", bufs=4) as ohp,
            tc.tile_pool(name="dump", bufs=4) as dumpp,
            tc.tile_pool(name="state", bufs=4) as statep,
            tc.tile_pool(name="small", bufs=6) as smallp,
            tc.tile_pool(name="tp_ps", bufs=2, space="PSUM") as tp_ps,
            tc.tile_pool(name="q_ps", bufs=4, space="PSUM") as q_ps,
            tc.tile_pool(name="c_ps", bufs=1, space="PSUM") as c_ps,
            tc.tile_pool(name="m_ps", bufs=1, space="PSUM") as m_ps,
        ):
            # ---------------- constants / prologue ----------------
            ident = consts.tile([128, 128], fp32)
            make_identity(nc, ident)
            ident_bf = consts.tile([128, 128], bf16)
            make_identity(nc, ident_bf)
            iota = consts.tile([128, 128], fp32)
            nc.gpsimd.iota(
                iota, pattern=[[1, 128]], base=0, channel_multiplier=0,
                allow_small_or_imprecise_dtypes=True,
            )
            ones_col_bf = consts.tile([128, 1], bf16)
            nc.vector.memset(ones_col_bf, 1.0)
            ones_col_f = consts.tile([128, 1], fp32)
            nc.vector.memset(ones_col_f, 1.0)
            ones_row_f = consts.tile([1, 128], fp32)
            nc.vector.memset(ones_row_f, 1.0)

            # transitions -> W = exp(trans) bf16, WT = W^T bf16
            tr_sb = consts.tile([128, 128], fp32)
            nc.sync.dma_start(out=tr_sb, in_=tr_d[:, :])
            w_bf = consts.tile([128, 128], bf16)
            nc.scalar.activation(w_bf, tr_sb, AF.Exp)
            wt_psum = tp_ps.tile([128, 128], bf16, tag="tp")
            nc.tensor.transpose(wt_psum, w_bf, ident_bf)
            wt_bf = consts.tile([128, 128], bf16)
            nc.vector.tensor_copy(wt_bf, wt_psum)

            # chat = mean_j ln(colsum_j W) over j=1..127  (col 0 is exp(-1e4)=0)
            colw_ps = m_ps.tile([1, 128], fp32, tag="m")
            nc.tensor.matmul(colw_ps, ones_col_bf, w_bf, start=True, stop=True)
            lncol = smallp.tile([1, 127], fp32, tag="lncol")
            lnsum = consts.tile([1, 1], fp32)
            nc.scalar.activation(lncol, colw_ps[:, 1:128], AF.Ln, accum_out=lnsum)
            chat_tot = consts.tile([1, 1], fp32)
            nc.scalar.activation(chat_tot, lnsum, AF.Copy, scale=float(S) / 127.0)
            negchat = consts.tile([1, 1], fp32)
            nc.scalar.activation(negchat, lnsum, AF.Copy, scale=-1.0 / 127.0)
            # broadcast -chat to [128,1] for use as activation bias
            nbc_ps = m_ps.tile([128, 1], fp32, tag="m")
            nc.tensor.matmul(nbc_ps, ones_row_f, negchat, start=True, stop=True)
            negchat_bc = consts.tile([128, 1], fp32)
            nc.vector.tensor_copy(negchat_bc, nbc_ps)

            # tags -> f32, transposed into [s(128), (sb,b)] column layout,
            # plus a shift-by-one variant for transition pairs
            tags_nat = consts.tile([BC, S], mybir.dt.int32)
            nc.sync.dma_start(out=tags_nat, in_=tg_d[:, :])
            tags_f = consts.tile([BC, S], fp32)
            nc.vector.tensor_copy(tags_f, tags_nat)
            tag_cols = consts.tile([128, NSB * BC], fp32)   # col = sb*16 + b
            tagsh_cols = consts.tile([128, NSB * BC], fp32)
            nc.vector.memset(tagsh_cols[:, (NSB - 1) * BC:], JUNK_TAG)
            for sb in range(NSB):
                tp = tp_ps.tile([128, BC], fp32, tag="tp")
                nc.tensor.transpose(
                    tp, tags_f[:, sb * 128:(sb + 1) * 128], ident[:BC, :BC]
                )
                nc.vector.tensor_copy(tag_cols[:, sb * BC:(sb + 1) * BC], tp)
            for sb in range(NSB):
                n = 128 if sb < NSB - 1 else 127
                tp = tp_ps.tile([128, BC], fp32, tag="tp")
                nc.tensor.transpose(
                    tp[:n], tags_f[:, sb * 128 + 1: sb * 128 + 1 + n],
                    ident[:BC, :BC],
                )
                nc.vector.tensor_copy(
                    tagsh_cols[:n, sb * BC:(sb + 1) * BC], tp[:n]
                )

            # ---------------- E precompute ----------------
            ebuf = ebufp.tile([128, S * BC], bf16)  # free index = t*16 + b
            ebuf3 = ebuf.rearrange("p (t b) -> p t b", b=BC)

            def emit_E(b, sb):
                emis = emisp.tile([128, 128], fp32, tag="emis")
                nc.sync.dma_start(
                    out=emis, in_=em_d[b, sb * 128:(sb + 1) * 128, :]
                )
                tp = tp_ps.tile([128, 128], fp32, tag="tp")
                nc.tensor.transpose(tp, emis, ident)
                # exp(x - chat) into strided slots: free = (sb*128+s)*16 + b
                nc.scalar.activation(
                    ebuf3[:, sb * 128:(sb + 1) * 128, b], tp, AF.Exp,
                    bias=negchat_bc,
                )

            # ---------------- gold path ----------------
            # esel_cols[s, b*16+sb]: per-block emission-select partials
            esel_cols = consts.tile([128, BC * NSB], fp32)
            tsum_cols = consts.tile([128, BC], fp32)  # per-b sum(C*trans)
            if gold_mode != 'all':
                nc.vector.memset(esel_cols, 0.0)
                nc.vector.memset(tsum_cols, 0.0)
            gold_c = [None]

            def emit_gold(b, sb):
                col = sb * BC + b
                oh = ohp.tile([128, 128], bf16, tag="oh")
                ohs = ohp.tile([128, 128], bf16, tag="ohs")
                if gold_mode == 'nooh':
                    nc.vector.memset(oh, 0.0)
                    nc.vector.memset(ohs, 0.0)
                else:
                    nc.gpsimd.tensor_scalar(
                        out=oh, in0=iota, scalar1=tag_cols[:, col:col + 1],
                        scalar2=None, op0=ALU.is_equal,
                    )
                    nc.gpsimd.tensor_scalar(
                        out=ohs, in0=iota, scalar1=tagsh_cols[:, col:col + 1],
                        scalar2=None, op0=ALU.is_equal,
                    )
                if gold_mode == 'ohonly':
                    return
                if sb == 0:
                    gold_c[0] = c_ps.tile(
                        [128, 128], fp32, tag="cps", name="gold_c"
                    )
                if gold_mode != 'nocmm':
                    if gold_mode == 'cmmsolo':
                        nc.tensor.matmul(gold_c[0], oh, ohs, start=True, stop=True)
                    else:
                        nc.tensor.matmul(
                            gold_c[0], oh, ohs,
                            start=(sb == 0), stop=(sb == NSB - 1),
                        )
                if gold_mode != 'nosel':
                    # emission select: sum_t emis[s, t] * oh[s, t]
                    emis2 = emis2p.tile([128, 128], fp32, tag="emis2")
                    nc.sync.dma_start(
                        out=emis2, in_=em_d[b, sb * 128:(sb + 1) * 128, :]
                    )
                    dump = dumpp.tile([128, 128], fp32, tag="gdump")
                    nc.gpsimd.tensor_tensor(out=dump, in0=emis2, in1=oh, op=ALU.mult)
                    nc.scalar.activation(
                        dump, dump, AF.Copy,
                        accum_out=esel_cols[:, b * NSB + sb: b * NSB + sb + 1],
                    )
                if gold_mode == 'nocmm':
                    return
                if sb == NSB - 1:
                    cdump = dumpp.tile([128, 128], fp32, tag="cdump")
                    nc.vector.tensor_tensor(
                        out=cdump, in0=gold_c[0], in1=tr_sb, op=ALU.mult
                    )
                    nc.scalar.activation(
                        cdump, cdump, AF.Copy,
                        accum_out=tsum_cols[:, b:b + 1],
                    )

            # ---------------- chain ----------------
            acc = consts.tile([1, 2 * BC], fp32)  # [fwd | bwd] halves
            nc.vector.memset(acc, 0.0)

            def renorm(v, which):
                """colsum -> reciprocal -> broadcast; accumulate ln into acc."""
                cs = m_ps.tile([1, BC], fp32, tag="m")
                nc.tensor.matmul(cs, ones_col_bf, v, start=True, stop=True)
                rec = smallp.tile([1, BC], fp32, tag="rec")
                nc.vector.reciprocal(rec, cs)
                lng = smallp.tile([1, BC], fp32, tag="lng")
                nc.scalar.activation(lng, cs, AF.Ln)
                bc_ps = m_ps.tile([128, BC], fp32, tag="m")
                nc.tensor.matmul(bc_ps, ones_row_f, rec, start=True, stop=True)
                a = acc[:, which * BC:(which + 1) * BC]
                nc.vector.tensor_tensor(out=a, in0=a, in1=lng, op=ALU.add)
                return bc_ps

            # side work queue: remaining E tiles (inner sblocks), then gold
            if nrot is None:
                nrot_lim = S - 1 - MID
            else:
                nrot_lim = nrot
            side = []
            order = [0, NSB - 1]
            for k in range(1, NSB // 2):
                order += [k, NSB - 1 - k]
            for sb in order[2:]:
                for b in range(BC):
                    side.append(("E", b, sb))
            if do_gold:
                for b in range(BC):
                    for sb in range(NSB):
                        side.append(("G", b, sb))
            else:
                nc.vector.memset(esel_cols, 0.0)
                nc.vector.memset(tsum_cols, 0.0)

            # prologue: E for first/last sblocks (chain start needs these)
            for sb in order[:2]:
                for b in range(BC):
                    emit_E(b, sb)

            def do_side(n):
                for _ in range(n):
                    if side:
                        kind, b, sb = side.pop(0)
                        if kind == "E":
                            emit_E(b, sb)
                        else:
                            emit_gold(b, sb)

            def eslice(t):
                return ebuf[:, t * BC:(t + 1) * BC]

            vf = eslice(0)          # a_0 = E_0
            vb = eslice(S - 1)      # y_{2047} = E_{2047}
            bc_f = None
            bc_b = None
            vb_fin = None
            NROT = S - 1 - MID      # 1024 rotations
            for r in range(NROT if do_chain else 0):
                if r >= nrot_lim:
                    break
                # forward step t = r+1:  a_t = (W^T a_{t-1}) * E_t   (lhsT=W)
                if r < MID:
                    t = r + 1
                    qf = q_ps.tile([128, BC], fp32, tag="q")
                    nc.tensor.matmul(qf, w_bf, vf, start=True, stop=True)
                    nvf = statep.tile([128, BC], bf16, tag="vf")
                    nc.vector.tensor_tensor(out=nvf, in0=qf, in1=eslice(t), op=ALU.mult)
                    if bc_f is not None:
                        nc.vector.tensor_tensor(out=nvf, in0=nvf, in1=bc_f, op=ALU.mult)
                        bc_f = None
                    vf = nvf
                    if t % RENORM == 0 and t < MID:
                        bc_f = renorm(vf, 0)
                # backward: q = W @ y_{t+1}; t from 2046 down to MID
                t = S - 2 - r
                qb = q_ps.tile([128, BC], fp32, tag="q")
                nc.tensor.matmul(qb, wt_bf, vb, start=True, stop=True)
                if t == MID:
                    vb_fin = qb  # b_MID = W y_{MID+1}: final, stays in PSUM
                else:
                    nvb = statep.tile([128, BC], bf16, tag="vb")
                    nc.vector.tensor_tensor(out=nvb, in0=qb, in1=eslice(t), op=ALU.mult)
                    if bc_b is not None:
                        nc.vector.tensor_tensor(out=nvb, in0=nvb, in1=bc_b, op=ALU.mult)
                        bc_b = None
                    vb = nvb
                    # the scale from a renorm at t is applied at step t-1, so
                    # the last chance is t == MID+2 (applied at MID+1)
                    if t % RENORM == 0 and t > MID + 1:
                        bc_b = renorm(vb, 1)
                do_side(1 if r % 2 == 0 else 0)

            do_side(len(side))
            if not do_chain or nrot_lim < S - 1 - MID:
                vvf = statep.tile([128, BC], bf16, tag="vf")
                nc.vector.memset(vvf, 1.0)
                vf = vvf
                vb_fin = q_ps.tile([128, BC], fp32, tag="q", name="vbfin")
                nc.tensor.matmul(vb_fin, wt_bf, vvf, start=True, stop=True)

            # ---------------- epilogue ----------------
            # log_Z = ln(sum_j vf*vb_fin) + acc_f + acc_b + S*chat
            dotd = dumpp.tile([128, BC], fp32, tag="dotd")
            nc.vector.tensor_tensor(out=dotd, in0=vb_fin, in1=vf, op=ALU.mult)
            zs = m_ps.tile([1, BC], fp32, tag="m")
            nc.tensor.matmul(zs, ones_col_f, dotd, start=True, stop=True)
            lnz = smallp.tile([1, BC], fp32, tag="lnz")
            nc.scalar.activation(lnz, zs, AF.Ln)
            logz = smallp.tile([1, BC], fp32, tag="logz")
            nc.vector.tensor_tensor(
                out=logz, in0=lnz, in1=acc[:, 0:BC], op=ALU.add
            )
            nc.vector.tensor_tensor(
                out=logz, in0=logz, in1=acc[:, BC:2 * BC], op=ALU.add
            )
            nc.vector.tensor_scalar(
                out=logz, in0=logz, scalar1=chat_tot, scalar2=None, op0=ALU.add
            )

            # seq score: emission part = grouped colsums of esel_cols,
            # transition part = colsums of tsum_cols
            es_ps = m_ps.tile([1, BC * NSB], fp32, tag="m")
            nc.tensor.matmul(es_ps, ones_col_f, esel_cols, start=True, stop=True)
            esel_b = smallp.tile([1, BC], fp32, tag="eselb")
            nc.vector.tensor_reduce(
                esel_b,
                es_ps.rearrange("p (b sb) -> p b sb", b=BC),
                axis=AX.X, op=ALU.add,
            )
            ts_ps = m_ps.tile([1, BC], fp32, tag="m")
            nc.tensor.matmul(ts_ps, ones_col_f, tsum_cols, start=True, stop=True)
            res = smallp.tile([1, BC], fp32, tag="res")
            nc.vector.tensor_tensor(out=res, in0=logz, in1=esel_b, op=ALU.subtract)
            nc.vector.tensor_tensor(out=res, in0=res, in1=ts_ps, op=ALU.subtract)
            nc.sync.dma_start(out=out_d[:], in_=res[0:1, :])
            dbg = smallp.tile([1, 4 * BC], fp32, tag="dbg")
            nc.vector.tensor_copy(dbg[:, 0:BC], logz)
            nc.vector.tensor_copy(dbg[:, BC:2 * BC], esel_b)
            nc.vector.tensor_copy(dbg[:, 2 * BC:3 * BC], ts_ps)
            nc.vector.tensor_copy(dbg[:, 3 * BC:4 * BC], lnz)
            nc.sync.dma_start(out=dbg_d[:], in_=dbg[0:1, :])

    return nc


def _get_compiled(finalized=False):
    global _compiled
    if _compiled is None:
        _compiled = _build_program()
    if finalized and not _compiled.is_finalized():
        _compiled.finalize()
    return _compiled


def make_in_maps(emissions, transitions, tags):
    in_maps = []
    for c in range(NCORES):
        sl = slice(c * BC, (c + 1) * BC)
        in_maps.append({
            "emissions_sh": np.ascontiguousarray(emissions[sl], dtype=np.float32),
            "transitions": np.ascontiguousarray(transitions, dtype=np.float32),
            "tags_sh": np.ascontiguousarray(tags[sl]).astype(np.int32),
        })
    return in_maps


def _run_device(emissions, transitions, tags):
    from concourse.bass_utils import run_bass_kernel_spmd

    nc = _get_compiled(finalized=True)
    res = run_bass_kernel_spmd(
        nc, make_in_maps(emissions, transitions, tags), list(range(NCORES))
    )
    parts = np.concatenate([res.results[c]["loss_parts"] for c in range(NCORES)])
    return np.float32(parts.mean())


def _run_host(emissions, transitions, tags, mask):
    """Slow but fully general fallback (any mask pattern)."""
    e = emissions.astype(np.float64)
    t = transitions.astype(np.float64)

    def lse(x, axis):
        m = x.max(axis=axis, keepdims=True)
        return (m + np.log(np.exp(x - m).sum(axis=axis, keepdims=True))).squeeze(axis)

    score = e[:, 0]
    for s in range(1, e.shape[1]):
        nxt = lse(score[:, :, None] + t[None, :, :] + e[:, s, None, :], axis=1)
        score = np.where(mask[:, s, None], nxt, score)
    log_Z = lse(score, axis=1)
    emit = np.take_along_axis(e, tags[..., None].astype(np.int64), axis=2)[..., 0]
    trans_sc = t[tags[:, :-1].astype(np.int64), tags[:, 1:].astype(np.int64)]
    m = mask[:, 1:].astype(np.float64)
    seq = emit[:, 0] + ((trans_sc + emit[:, 1:]) * m).sum(axis=1)
    return np.float32((log_Z - seq).mean())


def kernel(emissions, transitions, tags, mask):
    emissions = np.asarray(emissions)
    transitions = np.asarray(transitions)
    tags = np.asarray(tags)
    mask = np.asarray(mask)
    if emissions.shape != (B, S, T) or not mask.all():
        return _run_host(emissions, transitions, tags, mask)
    return _run_device(emissions, transitions, tags)


# revision 20
# speedup vs baseline: 2.2746x; 2.2746x over previous
"""Trainium2 Bass kernel for a batched linear-chain CRF negative log-likelihood.

reference semantics (B=128, S=2048, T=128):
    forward algorithm over S steps (log-space matvec chain) -> log_Z per batch
    gold path score = emissions gathered at tags + transitions gathered at
    (tag_t, tag_{t+1}) pairs, summed over time
    output = mean(log_Z - seq_score)   (scalar f32)

Strategy:
  - data parallel over 8 cores: 16 batch rows per core, transitions replicated.
  - linear space: a_t = (a_{t-1} @ W) * E_t with W = exp(transitions),
    E_t = exp(emit_t - chat).  Per-step work: one PE matmul (stationary W,
    moving state [128 tags x 16 batch]) + one DVE multiply out of PSUM.
  - bidirectional: forward chain from t=0 and a backward chain
    y_t = E_t * (W @ y_{t+1}) from t=2047 run concurrently and meet at
    t=1023: log_Z = log(a_m . (W y_{m+1})) + accumulated log scales.
  - periodic renormalization (every 32 steps) keeps values in range; scale
    logs accumulate per batch.
  - E precomputed into a transposed [tag, t*16+b] bf16 buffer via PE
    transpose + scalar-engine exp evacuation (bias = -chat).
  - gold path on device: one-hot tiles (iota vs tag compare on gpsimd),
    pair-count matrix C_b = OH^T @ OHshift accumulated on PE,
    sum(C*trans) via DVE fused tensor_tensor_reduce; emission select via
    gpsimd multiply + scalar-engine copy-with-accumulate.
"""

import numpy as np

B, S, T = 128, 2048, 128
NCORES = 8
BC = B // NCORES  # 16 batch rows per core
NSB = S // 128  # 16 s-blocks of 128
MID = S // 2 - 1  # 1023: chains meet here
RENORM = 32
JUNK_TAG = 60000.0  # one-hot of this is all zeros (tags are < 128)

_compiled = None


def _build_program(do_chain=True, do_gold=True, nrot=None, gold_mode='all'):
    import concourse.bass as bass
    import concourse.bacc as bacc
    import concourse.tile as tile
    from concourse import mybir
    from concourse.masks import make_identity

    fp32 = mybir.dt.float32
    bf16 = mybir.dt.bfloat16
    AF = mybir.ActivationFunctionType
    ALU = mybir.AluOpType
    AX = mybir.AxisListType

    nc = bacc.Bacc(None)
    em_d = nc.declare_dram_parameter("emissions_sh", [BC, S, T], fp32, isOutput=False)
    tr_d = nc.declare_dram_parameter("transitions", [T, T], fp32, isOutput=False)
    tg_d = nc.declare_dram_parameter("tags_sh", [BC, S], mybir.dt.int32, isOutput=False)
    out_d = nc.declare_dram_parameter("loss_parts", [BC], fp32, isOutput=True)
    dbg_d = nc.declare_dram_parameter("dbg", [4 * BC], fp32, isOutput=True)

    with tile.TileContext(nc) as tc:
        with (
            tc.tile_pool(name="consts", bufs=1) as consts,
            tc.tile_pool(name="ebuf", bufs=1) as ebufp,
            tc.tile_pool(name="emis", bufs=4) as emisp,
            tc.tile_pool(name="emis2", bufs=4) as emis2p,
            tc.tile_pool(name="oh", bufs=4) as ohp,
            tc.tile_pool(name="dump", bufs=4) as dumpp,
            tc.tile_pool(name="state", bufs=4) as statep,
            tc.tile_pool(name="small", bufs=6) as smallp,
            tc.tile_pool(name="tp_ps", bufs=2, space="PSUM") as tp_ps,
            tc.tile_pool(name="q_ps", bufs=4, space="PSUM") as q_ps,
            tc.tile_pool(name="c_ps", bufs=1, space="PSUM") as c_ps,
            tc.tile_pool(name="m_ps", bufs=1, space="PSUM") as m_ps,
        ):
            # ---------------- constants / prologue ----------------
            ident = consts.tile([128, 128], fp32)
            make_identity(nc, ident)
            ident_bf = consts.tile([128, 128], bf16)
            make_identity(nc, ident_bf)
            iota = consts.tile([128, 128], bf16)
            nc.gpsimd.iota(
                iota, pattern=[[1, 128]], base=0, channel_multiplier=0,
                allow_small_or_imprecise_dtypes=True,
            )
            ones_col_bf = consts.tile([128, 1], bf16)
            nc.vector.memset(ones_col_bf, 1.0)
            ones_col_f = consts.tile([128, 1], fp32)
            nc.vector.memset(ones_col_f, 1.0)
            ones_row_f = consts.tile([1, 128], fp32)
            nc.vector.memset(ones_row_f, 1.0)

            # transitions -> W = exp(trans) bf16, WT = W^T bf16
            tr_sb = consts.tile([128, 128], fp32)
            nc.sync.dma_start(out=tr_sb, in_=tr_d[:, :])
            w_bf = consts.tile([128, 128], bf16)
            nc.scalar.activation(w_bf, tr_sb, AF.Exp)
            wt_psum = tp_ps.tile([128, 128], bf16, tag="tp")
            nc.tensor.transpose(wt_psum, w_bf, ident_bf)
            wt_bf = consts.tile([128, 128], bf16)
            nc.vector.tensor_copy(wt_bf, wt_psum)

            # chat = mean_j ln(colsum_j W) over j=1..127  (col 0 is exp(-1e4)=0)
            colw_ps = m_ps.tile([1, 128], fp32, tag="m")
            nc.tensor.matmul(colw_ps, ones_col_bf, w_bf, start=True, stop=True)
            lncol = smallp.tile([1, 127], fp32, tag="lncol")
            lnsum = consts.tile([1, 1], fp32)
            nc.scalar.activation(lncol, colw_ps[:, 1:128], AF.Ln, accum_out=lnsum)
            chat_tot = consts.tile([1, 1], fp32)
            nc.scalar.activation(chat_tot, lnsum, AF.Copy, scale=float(S) / 127.0)
            negchat = consts.tile([1, 1], fp32)
            nc.scalar.activation(negchat, lnsum, AF.Copy, scale=-1.0 / 127.0)
            # broadcast -chat to [128,1] for use as activation bias
            nbc_ps = m_ps.tile([128, 1], fp32, tag="m")
            nc.tensor.matmul(nbc_ps, ones_row_f, negchat, start=True, stop=True)
            negchat_bc = consts.tile([128, 1], fp32)
            nc.vector.tensor_copy(negchat_bc, nbc_ps)

            # tags -> f32, transposed into [s(128), (sb,b)] column layout,
            # plus a shift-by-one variant for transition pairs
            tags_nat = consts.tile([BC, S], mybir.dt.int32)
            nc.sync.dma_start(out=tags_nat, in_=tg_d[:, :])
            tags_f = consts.tile([BC, S], fp32)
            nc.vector.tensor_copy(tags_f, tags_nat)
            tag_cols = consts.tile([128, NSB * BC], fp32)   # col = sb*16 + b
            tagsh_cols = consts.tile([128, NSB * BC], fp32)
            nc.vector.memset(tagsh_cols[:, (NSB - 1) * BC:], JUNK_TAG)
            for sb in range(NSB):
                tp = tp_ps.tile([128, BC], fp32, tag="tp")
                nc.tensor.transpose(
                    tp, tags_f[:, sb * 128:(sb + 1) * 128], ident[:BC, :BC]
                )
                nc.vector.tensor_copy(tag_cols[:, sb * BC:(sb + 1) * BC], tp)
            for sb in range(NSB):
                n = 128 if sb < NSB - 1 else 127
                tp = tp_ps.tile([128, BC], fp32, tag="tp")
                nc.tensor.transpose(
                    tp[:n], tags_f[:, sb * 128 + 1: sb * 128 + 1 + n],
                    ident[:BC, :BC],
                )
                nc.vector.tensor_copy(
                    tagsh_cols[:n, sb * BC:(sb + 1) * BC], tp[:n]
                )

            # ---------------- E precompute ----------------
            ebuf = ebufp.tile([128, S * BC], bf16)  # free index = b*S + t
            ebuf3 = ebuf.rearrange("p (b t) -> p b t", t=S)

            def emit_E(b, sb):
                emis = emisp.tile([128, 128], fp32, tag="emis")
                nc.sync.dma_start(
                    out=emis, in_=em_d[b, sb * 128:(sb + 1) * 128, :]
                )
                tp = tp_ps.tile([128, 128], fp32, tag="tp")
                nc.tensor.transpose(tp, emis, ident)
                # exp(x - chat), contiguous run: free = b*S + sb*128 + s
                nc.scalar.activation(
                    ebuf3[:, b, sb * 128:(sb + 1) * 128], tp, AF.Exp,
                    bias=negchat_bc,
                )

            # ---------------- gold path ----------------
            # esel_cols[s, b*16+sb]: per-block emission-select partials
            esel_cols = consts.tile([128, BC * NSB], fp32)
            tsum_cols = consts.tile([128, BC], fp32)  # per-b sum(C*trans)
            if gold_mode != 'all':
                nc.vector.memset(esel_cols, 0.0)
                nc.vector.memset(tsum_cols, 0.0)
            gold_c = [None]

            def emit_gold(b, sb):
                col = sb * BC + b
                oh = ohp.tile([128, 128], bf16, tag="oh")
                ohs = ohp.tile([128, 128], bf16, tag="ohs")
                if gold_mode == 'nooh':
                    nc.vector.memset(oh, 0.0)
                    nc.vector.memset(ohs, 0.0)
                else:
                    nc.vector.tensor_scalar(
                        out=oh, in0=iota, scalar1=tag_cols[:, col:col + 1],
                        scalar2=None, op0=ALU.is_equal,
                    )
                    nc.vector.tensor_scalar(
                        out=ohs, in0=iota, scalar1=tagsh_cols[:, col:col + 1],
                        scalar2=None, op0=ALU.is_equal,
                    )
                if gold_mode == 'ohonly':
                    return
                if sb == 0:
                    gold_c[0] = c_ps.tile(
                        [128, 128], fp32, tag="cps", name="gold_c"
                    )
                if gold_mode != 'nocmm':
                    if gold_mode == 'cmmsolo':
                        nc.tensor.matmul(gold_c[0], oh, ohs, start=True, stop=True)
                    else:
                        nc.tensor.matmul(
                            gold_c[0], oh, ohs,
                            start=(sb == 0), stop=(sb == NSB - 1),
                        )
                if gold_mode != 'nosel':
                    # emission select: sum_t emis[s, t] * oh[s, t]
                    emis2 = emis2p.tile([128, 128], fp32, tag="emis2")
                    nc.sync.dma_start(
                        out=emis2, in_=em_d[b, sb * 128:(sb + 1) * 128, :]
                    )
                    dump = dumpp.tile([128, 128], fp32, tag="gdump")
                    nc.gpsimd.tensor_tensor(out=dump, in0=emis2, in1=oh, op=ALU.mult)
                    nc.scalar.activation(
                        dump, dump, AF.Copy,
                        accum_out=esel_cols[:, b * NSB + sb: b * NSB + sb + 1],
                    )
                if gold_mode == 'nocmm':
                    return
                if sb == NSB - 1:
                    cdump = dumpp.tile([128, 128], fp32, tag="cdump")
                    nc.vector.tensor_tensor(
                        out=cdump, in0=gold_c[0], in1=tr_sb, op=ALU.mult
                    )
                    nc.scalar.activation(
                        cdump, cdump, AF.Copy,
                        accum_out=tsum_cols[:, b:b + 1],
                    )

            # ---------------- chain ----------------
            # raw renorm colsums parked here (b-major, 64 slots per b);
            # ln of all of them is taken once in the epilogue
            NRE = 64
            glog = consts.tile([1, BC * NRE], fp32)
            nc.vector.memset(glog, 1.0)
            glog3 = glog.rearrange("p (b k) -> p b k", k=NRE)
            renorm_k = [0]

            def renorm(v, which):
                """colsum -> reciprocal -> broadcast; park colsum for epilogue."""
                cs = m_ps.tile([1, BC], fp32, tag="m")
                nc.tensor.matmul(cs, ones_col_bf, v, start=True, stop=True)
                rec = smallp.tile([1, BC], fp32, tag="rec")
                nc.vector.reciprocal(rec, cs)
                k = renorm_k[0]
                renorm_k[0] += 1
                nc.vector.tensor_copy(glog3[:, :, k], cs)
                bc_ps = m_ps.tile([128, BC], fp32, tag="m")
                nc.tensor.matmul(bc_ps, ones_row_f, rec, start=True, stop=True)
                return bc_ps

            # side work queue: remaining E tiles (inner sblocks), then gold
            if nrot is None:
                nrot_lim = S - 1 - MID
            else:
                nrot_lim = nrot
            side = []
            order = [0, NSB - 1]
            for k in range(1, NSB // 2):
                order += [k, NSB - 1 - k]
            for sb in order[2:]:
                for b in range(BC):
                    side.append(("E", b, sb))
            if do_gold:
                for b in range(BC):
                    for sb in range(NSB):
                        side.append(("G", b, sb))
            else:
                nc.vector.memset(esel_cols, 0.0)
                nc.vector.memset(tsum_cols, 0.0)

            # prologue: E for first/last sblocks (chain start needs these)
            for sb in order[:2]:
                for b in range(BC):
                    emit_E(b, sb)

            def do_side(n):
                for _ in range(n):
                    if side:
                        kind, b, sb = side.pop(0)
                        if kind == "E":
                            emit_E(b, sb)
                        else:
                            emit_gold(b, sb)

            def eslice(t):
                return ebuf3[:, :, t]

            vf = eslice(0)          # a_0 = E_0
            vb = eslice(S - 1)      # y_{2047} = E_{2047}
            bc_f = None
            bc_b = None
            vb_fin = None
            NROT = S - 1 - MID      # 1024 rotations
            for r in range(NROT if do_chain else 0):
                if r >= nrot_lim:
                    break
                # forward step t = r+1:  a_t = (W^T a_{t-1}) * E_t   (lhsT=W)
                if r < MID:
                    t = r + 1
                    qf = q_ps.tile([128, BC], fp32, tag="q")
                    nc.tensor.matmul(qf, w_bf, vf, start=True, stop=True)
                    nvf = statep.tile([128, BC], bf16, tag="vf")
                    nc.vector.tensor_tensor(out=nvf, in0=qf, in1=eslice(t), op=ALU.mult)
                    if bc_f is not None:
                        nc.vector.tensor_tensor(out=nvf, in0=nvf, in1=bc_f, op=ALU.mult)
                        bc_f = None
                    vf = nvf
                    if (t % RENORM == 0 or t == 1008) and t < MID:
                        bc_f = renorm(vf, 0)
                # backward: q = W @ y_{t+1}; t from 2046 down to MID
                t = S - 2 - r
                qb = q_ps.tile([128, BC], fp32, tag="q")
                nc.tensor.matmul(qb, wt_bf, vb, start=True, stop=True)
                if t == MID:
                    vb_fin = qb  # b_MID = W y_{MID+1}: final, stays in PSUM
                else:
                    nvb = statep.tile([128, BC], bf16, tag="vb")
                    nc.vector.tensor_tensor(out=nvb, in0=qb, in1=eslice(t), op=ALU.mult)
                    if bc_b is not None:
                        nc.vector.tensor_tensor(out=nvb, in0=nvb, in1=bc_b, op=ALU.mult)
                        bc_b = None
                    vb = nvb
                    # the scale from a renorm at t is applied at step t-1, so
                    # the last chance is t == MID+2 (applied at MID+1)
                    if (t % RENORM == 0 or t == 1040) and t > MID + 1:
                        bc_b = renorm(vb, 1)
                do_side(1 if r % 2 == 0 else 0)

            do_side(len(side))
            if not do_chain or nrot_lim < S - 1 - MID:
                vvf = statep.tile([128, BC], bf16, tag="vf")
                nc.vector.memset(vvf, 1.0)
                vf = vvf
                vb_fin = q_ps.tile([128, BC], fp32, tag="q", name="vbfin")
                nc.tensor.matmul(vb_fin, wt_bf, vvf, start=True, stop=True)

            # ---------------- epilogue ----------------
            # log_Z = ln(sum_j vf*vb_fin) + acc_f + acc_b + S*chat
            dotd = dumpp.tile([128, BC], fp32, tag="dotd")
            nc.vector.tensor_tensor(out=dotd, in0=vb_fin, in1=vf, op=ALU.mult)
            zs = m_ps.tile([1, BC], fp32, tag="m")
            nc.tensor.matmul(zs, ones_col_f, dotd, start=True, stop=True)
            lnz = smallp.tile([1, BC], fp32, tag="lnz")
            nc.scalar.activation(lnz, zs, AF.Ln)
            lnglog = smallp.tile([1, BC * NRE], fp32, tag="lnglog")
            nc.scalar.activation(lnglog, glog, AF.Ln)
            accsum = smallp.tile([1, BC], fp32, tag="accsum")
            nc.vector.tensor_reduce(
                accsum,
                lnglog.rearrange("p (b k) -> p b k", k=NRE),
                axis=AX.X, op=ALU.add,
            )
            logz = smallp.tile([1, BC], fp32, tag="logz")
            nc.vector.tensor_tensor(out=logz, in0=lnz, in1=accsum, op=ALU.add)
            nc.vector.tensor_scalar(
                out=logz, in0=logz, scalar1=chat_tot, scalar2=None, op0=ALU.add
            )

            # seq score: emission part = grouped colsums of esel_cols,
            # transition part = colsums of tsum_cols
            es_ps = m_ps.tile([1, BC * NSB], fp32, tag="m")
            nc.tensor.matmul(es_ps, ones_col_f, esel_cols, start=True, stop=True)
            esel_b = smallp.tile([1, BC], fp32, tag="eselb")
            nc.vector.tensor_reduce(
                esel_b,
                es_ps.rearrange("p (b sb) -> p b sb", b=BC),
                axis=AX.X, op=ALU.add,
            )
            ts_ps = m_ps.tile([1, BC], fp32, tag="m")
            nc.tensor.matmul(ts_ps, ones_col_f, tsum_cols, start=True, stop=True)
            res = smallp.tile([1, BC], fp32, tag="res")
            nc.vector.tensor_tensor(out=res, in0=logz, in1=esel_b, op=ALU.subtract)
            nc.vector.tensor_tensor(out=res, in0=res, in1=ts_ps, op=ALU.subtract)
            nc.sync.dma_start(out=out_d[:], in_=res[0:1, :])
            dbg = smallp.tile([1, 4 * BC], fp32, tag="dbg")
            nc.vector.tensor_copy(dbg[:, 0:BC], logz)
            nc.vector.tensor_copy(dbg[:, BC:2 * BC], esel_b)
            nc.vector.tensor_copy(dbg[:, 2 * BC:3 * BC], ts_ps)
            nc.vector.tensor_copy(dbg[:, 3 * BC:4 * BC], lnz)
            nc.sync.dma_start(out=dbg_d[:], in_=dbg[0:1, :])

    return nc


def _get_compiled(finalized=False):
    global _compiled
    if _compiled is None:
        _compiled = _build_program()
    if finalized and not _compiled.is_finalized():
        _compiled.finalize()
    return _compiled


def make_in_maps(emissions, transitions, tags):
    in_maps = []
    for c in range(NCORES):
        sl = slice(c * BC, (c + 1) * BC)
        in_maps.append({
            "emissions_sh": np.ascontiguousarray(emissions[sl], dtype=np.float32),
            "transitions": np.ascontiguousarray(transitions, dtype=np.float32),
            "tags_sh": np.ascontiguousarray(tags[sl]).astype(np.int32),
        })
    return in_maps


def _run_device(emissions, transitions, tags):
    from concourse.bass_utils import run_bass_kernel_spmd

    nc = _get_compiled(finalized=True)
    res = run_bass_kernel_spmd(
        nc, make_in_maps(emissions, transitions, tags), list(range(NCORES))
    )
    parts = np.concatenate([res.results[c]["loss_parts"] for c in range(NCORES)])
    return np.float32(parts.mean())


def _run_host(emissions, transitions, tags, mask):
    """Slow but fully general fallback (any mask pattern)."""
    e = emissions.astype(np.float64)
    t = transitions.astype(np.float64)

    def lse(x, axis):
        m = x.max(axis=axis, keepdims=True)
        return (m + np.log(np.exp(x - m).sum(axis=axis, keepdims=True))).squeeze(axis)

    score = e[:, 0]
    for s in range(1, e.shape[1]):
        nxt = lse(score[:, :, None] + t[None, :, :] + e[:, s, None, :], axis=1)
        score = np.where(mask[:, s, None], nxt, score)
    log_Z = lse(score, axis=1)
    emit = np.take_along_axis(e, tags[..., None].astype(np.int64), axis=2)[..., 0]
    trans_sc = t[tags[:, :-1].astype(np.int64), tags[:, 1:].astype(np.int64)]
    m = mask[:, 1:].astype(np.float64)
    seq = emit[:, 0] + ((trans_sc + emit[:, 1:]) * m).sum(axis=1)
    return np.float32((log_Z - seq).mean())


def kernel(emissions, transitions, tags, mask):
    emissions = np.asarray(emissions)
    transitions = np.asarray(transitions)
    tags = np.asarray(tags)
    mask = np.asarray(mask)
    if emissions.shape != (B, S, T) or not mask.all():
        return _run_host(emissions, transitions, tags, mask)
    return _run_device(emissions, transitions, tags)
